# revision 28
# baseline (speedup 1.0000x reference)
import os
import sys

import numpy as np

sys.path.insert(0, "/opt/trn_rl_repo")

import ml_dtypes  # noqa: E402

BF16 = ml_dtypes.bfloat16

# nn_AttentiveDecisionTree: B=4096, F=256, M=32, H=4, K=2, N_TREES=256, DEPTH=6, UNITS=16
B, F = 4096, 256
M = 32
H, K = 4, 2
N_TREES, DEPTH, UNITS = 256, 6, 16
L = 2 ** DEPTH
NCORES = 8
BL = B // NCORES          # 512 batch columns per core
BT = BL // 128            # 4 batch tiles of 128
S = M + 1                 # 33 attention rows
HK = H * K                # 8
SHK = S * HK              # 264
QK = HK + SHK             # 272 merged q0|kk columns
NCH = N_TREES // 128      # 2 tree chunks of 128

_PROG = None              # cached compiled Bass program
LAST_EXEC_NS = None       # filled when KBENCH_TRACE=1


def _sparsemax(z):
    d = z.shape[-1]
    z_sorted = np.sort(z, axis=-1)[..., ::-1]
    rng = np.arange(1, d + 1, dtype=z.dtype)
    cssv = np.cumsum(z_sorted, axis=-1).astype(z.dtype) - np.float32(1.0)
    support = (z_sorted - cssv / rng) > 0
    k = np.sum(support, axis=-1).astype(np.int32)
    tau = np.take_along_axis(cssv, (k - 1)[..., None], axis=-1) / k[..., None].astype(z.dtype)
    return np.maximum(z - tau, np.float32(0.0))


def _build_program():
    from contextlib import ExitStack

    import concourse.bacc as bacc
    import concourse.tile as tile
    from concourse import mybir

    f32 = mybir.dt.float32
    bf16 = mybir.dt.bfloat16
    AF = mybir.ActivationFunctionType
    ALU = mybir.AluOpType
    AX = mybir.AxisListType

    nc = bacc.Bacc(trn_type="TRN2", num_devices=NCORES, enable_asserts=False)

    d_xta = nc.dram_tensor("xta", [F + 1, BL], bf16, kind="ExternalInput")
    d_wqk = nc.dram_tensor("wqk", [F + 1, QK], bf16, kind="ExternalInput")
    d_wv = nc.dram_tensor("wv", [F + 1, SHK], bf16, kind="ExternalInput")
    d_wo = nc.dram_tensor("wo", [HK + 1, F], bf16, kind="ExternalInput")
    d_fs = nc.dram_tensor("fsw", [128, DEPTH, 2, NCH, 128], bf16, kind="ExternalInput")
    d_cb = nc.dram_tensor("cb", [128, NCH * DEPTH], f32, kind="ExternalInput")
    d_rw = nc.dram_tensor("rw", [128, NCH, 8, 8, UNITS], bf16, kind="ExternalInput")
    d_id = nc.dram_tensor("ident", [128, 128], f32, kind="ExternalInput")
    d_out = nc.dram_tensor("out", [UNITS, BL], f32, kind="ExternalOutput")

    with tile.TileContext(nc) as tc, ExitStack() as ctx:
        cp = ctx.enter_context(tc.tile_pool(name="const", bufs=1))

        # --- PE warmup: dense dummy matmuls while input DMAs run ---------
        # (HAM clock gate releases only after ~3.4us of sustained PE work)
        wu_l = cp.tile([128, 128], f32, tag="wu_l", name="wu_l")
        wu_r = cp.tile([128, BL], f32, tag="wu_r", name="wu_r")
        nc.vector.memset(wu_l[:], 0.0)
        nc.vector.memset(wu_r[:], 0.0)
        with tc.tile_pool(name="pwu", bufs=2, space="PSUM") as pwu:
            for w in range(6):
                wup = pwu.tile([128, BL], f32, space="PSUM", tag="wup", name="wup")
                nc.tensor.matmul(wup[:], wu_l[:], wu_r[:], start=True, stop=True)
        # preload the ACT spline table (exp set) during the DMA window
        wu_a = cp.tile([1, 8], f32, tag="wu_a", name="wu_a")
        nc.scalar.activation(wu_a[:], wu_l[0:1, 0:8], AF.Exp)

        # --- persistent tiles + input DMAs (spread across queues) --------
        xta0 = cp.tile([128, BL], bf16, tag="xta0", name="xta0")
        xta1 = cp.tile([128, BL], bf16, tag="xta1", name="xta1")
        wqk0 = cp.tile([128, QK], bf16, tag="wqk0", name="wqk0")
        wqk1 = cp.tile([128, QK], bf16, tag="wqk1", name="wqk1")
        wqkb = cp.tile([1, QK], bf16, tag="wqkb", name="wqkb")
        wv0 = cp.tile([128, SHK], bf16, tag="wv0", name="wv0")
        wv1 = cp.tile([128, SHK], bf16, tag="wv1", name="wv1")
        wvb = cp.tile([1, SHK], bf16, tag="wvb", name="wvb")
        xbon = cp.tile([1, BL], bf16, tag="xbon", name="xbon")
        nc.sync.dma_start(xta0[:], d_xta.ap()[0:128, :])
        nc.scalar.dma_start(wqk0[:], d_wqk.ap()[0:128, :])
        nc.gpsimd.dma_start(xta1[:], d_xta.ap()[128:256, :])
        nc.sync.dma_start(wv0[:], d_wv.ap()[0:128, :])
        nc.scalar.dma_start(wqk1[:], d_wqk.ap()[128:256, :])
        nc.gpsimd.dma_start(xbon[:], d_xta.ap()[256:257, :])
        nc.sync.dma_start(wv1[:], d_wv.ap()[128:256, :])
        nc.scalar.dma_start(wqkb[:], d_wqk.ap()[256:257, :])
        nc.sync.dma_start(wvb[:], d_wv.ap()[256:257, :])

        wo_m = cp.tile([HK, F], bf16, tag="wo_m", name="wo_m")
        wo_b = cp.tile([1, F], bf16, tag="wo_b", name="wo_b")
        nc.gpsimd.dma_start(wo_m[:], d_wo.ap()[0:HK, :])
        nc.gpsimd.dma_start(wo_b[:], d_wo.ap()[HK:HK + 1, :])
        id_s = cp.tile([128, 128], f32, tag="id_s", name="id_s")
        nc.gpsimd.dma_start(id_s[:], d_id.ap()[:])
        fs_s = cp.tile([128, DEPTH, 2, NCH, 128], bf16, tag="fs_s", name="fs_s")
        nc.gpsimd.dma_start(fs_s[:], d_fs.ap()[:])
        cb_s = cp.tile([128, NCH * DEPTH], f32, tag="cb_s", name="cb_s")
        nc.scalar.dma_start(cb_s[:], d_cb.ap()[:])
        rw_s = cp.tile([128, NCH, 8, 8, UNITS], bf16, tag="rw_s", name="rw_s")
        nc.scalar.dma_start(rw_s[:], d_rw.ap()[:])

        ot = cp.tile([HK, BL], bf16, tag="ot", name="ot")
        xt2h = cp.tile([128, 2, BL], bf16, tag="xt2h", name="xt2h")
        bins0 = cp.tile([128, DEPTH, 2, BL], bf16, tag="bins0", name="bins0")
        bins1 = cp.tile([128, DEPTH, 2, BL], bf16, tag="bins1", name="bins1")
        binsall = [bins0, bins1]
        outs = cp.tile([UNITS, BL], f32, tag="outs", name="outs")

        # --- attention: per 128-batch tile -------------------------------
        with tc.tile_pool(name="pqk", bufs=3, space="PSUM") as pqk, \
                tc.tile_pool(name="ptr", bufs=2, space="PSUM") as ptr, \
                tc.tile_pool(name="asb", bufs=3) as asb:
            for i in range(BT):
                bs = slice(i * 128, (i + 1) * 128)
                qvp = pqk.tile([128, 1024], f32, space="PSUM", tag="qvp", name="qvp")
                qkp = qvp[:, 0:QK]
                vvp = qvp[:, 512:512 + SHK]
                nc.tensor.matmul(qkp, xta0[:, bs], wqk0[:], start=True, stop=False)
                nc.tensor.matmul(qkp, xta1[:, bs], wqk1[:], start=False, stop=False)
                nc.tensor.matmul(qkp, xbon[:, bs], wqkb[:], start=False, stop=True)
                nc.tensor.matmul(vvp, xta0[:, bs], wv0[:], start=True, stop=False)
                nc.tensor.matmul(vvp, xta1[:, bs], wv1[:], start=False, stop=False)
                nc.tensor.matmul(vvp, xbon[:, bs], wvb[:], start=False, stop=True)

                qvs = asb.tile([128, 2, QK], bf16, tag="qvs", name="qvs")
                qv_v = qvp.rearrange("p (j c) -> p j c", j=2, c=512)[:, :, 0:QK]
                nc.scalar.copy(qvs[:], qv_v)
                vvs = qvs[:, 1, 0:SHK]

                kks_v = qvs[:, 0, HK:QK].rearrange("p (t h k) -> p t h k", t=S, h=H, k=K)
                q0_v = (qvs[:, 0, 0:HK].rearrange("p (h k) -> p h k", h=H, k=K)
                        .unsqueeze(1).broadcast_to([128, S, H, K]))
                tmp = asb.tile([128, S, H, K], bf16, tag="tmp", name="tmp")
                nc.vector.tensor_mul(tmp[:], kks_v, q0_v)
                scr = asb.tile([128, S, H], f32, tag="scr", name="scr")
                nc.vector.tensor_add(scr[:], tmp[:, :, :, 0], tmp[:, :, :, 1])
                e = asb.tile([128, S, H], bf16, tag="e", name="e")
                nc.scalar.activation(e[:], scr[:], AF.Exp)
                ssum = asb.tile([128, H], f32, tag="ssum", name="ssum")
                nc.vector.tensor_reduce(ssum[:], e[:].rearrange("p t h -> p h t"),
                                        axis=AX.X, op=ALU.add)
                rinv = asb.tile([128, H], f32, tag="rinv", name="rinv")
                nc.vector.reciprocal(rinv[:], ssum[:])
                vvs_v = vvs.rearrange("p (t h k) -> p t h k", t=S, h=H, k=K)
                e_v = e[:].unsqueeze(3).broadcast_to([128, S, H, K])
                tmp2 = asb.tile([128, S, H, K], bf16, tag="tmp2", name="tmp2")
                nc.vector.tensor_mul(tmp2[:], vvs_v, e_v)
                osum = asb.tile([128, H, K], f32, tag="osum", name="osum")
                nc.vector.tensor_reduce(osum[:], tmp2[:].rearrange("p t h k -> p h k t"),
                                        axis=AX.X, op=ALU.add)
                o = asb.tile([128, H, K], f32, tag="o", name="o")
                rinv_v = rinv[:].unsqueeze(2).broadcast_to([128, H, K])
                nc.vector.tensor_mul(o[:], osum[:], rinv_v)

                trp = ptr.tile([HK, 128], f32, space="PSUM", tag="trp", name="trp")
                nc.tensor.transpose(trp[:], o[:].rearrange("p h k -> p (h k)"), id_s[:])
                nc.scalar.copy(ot[0:HK, bs], trp[:])

        # PE filler matmuls: no data deps, pace via stream order; keep the
        # HAM clock gate released across DVE/ACT-bound stretches
        pdum = ctx.enter_context(tc.tile_pool(name="pdum", bufs=2, space="PSUM"))

        def dummy_mms(n, tag):
            for w in range(n):
                dmp = pdum.tile([UNITS, BL], f32, space="PSUM", tag="dmp",
                                name=f"dmp_{tag}_{w}")
                nc.tensor.matmul(dmp[:], rw_s[:, 0, 0, 0, :], xta0[:],
                                 start=True, stop=True)

        # --- x2 = x + attn_out (transposed layout), cast bf16 ------------
        with tc.tile_pool(name="pxh", bufs=2, space="PSUM") as pxh:
            dummy_mms(8, "preW")
            for fc in range(2):
                fsl = slice(fc * 128, (fc + 1) * 128)
                xh = pxh.tile([128, BL], f32, space="PSUM", tag="xh", name="xh")
                nc.tensor.matmul(xh[:], wo_m[:, fsl], ot[:], start=True, stop=False)
                nc.tensor.matmul(xh[:], wo_b[:, fsl], xbon[:], start=False, stop=True)
                xt_fc = xta0 if fc == 0 else xta1
                nc.vector.tensor_add(xt2h[:, fc, :], xh[:], xt_fc[:])
            dummy_mms(12, "postW")

        # --- ODT: feature matmuls + bins + leaf + response ---------------
        with tc.tile_pool(name="pfe", bufs=4, space="PSUM") as pfe, \
                tc.tile_pool(name="pout", bufs=1, space="PSUM") as pout, \
                tc.tile_pool(name="osb", bufs=2) as osb, \
                tc.tile_pool(name="plf", bufs=3) as plf:
            for nc2 in range(NCH):
                for d in range(DEPTH):
                    fp = pfe.tile([128, BL], f32, space="PSUM", tag="fp", name="fp")
                    nc.tensor.matmul(fp[:], fs_s[:, d, 0, nc2, :], xt2h[:, 0, :],
                                     start=True, stop=False)
                    nc.tensor.matmul(fp[:], fs_s[:, d, 1, nc2, :], xt2h[:, 1, :],
                                     start=False, stop=True)
                    idx = nc2 * DEPTH + d
                    r1 = osb.tile([128, BL], bf16, tag="r1", name="r1")
                    # r1 = relu(feat + c)
                    nc.scalar.activation(r1[:], fp[:], AF.Relu,
                                         bias=cb_s[:, idx:idx + 1], scale=1.0)
                    # bins = min(r1, 1) == clip01(feat + c)
                    nc.vector.tensor_scalar_min(binsall[nc2][:, d, 1, :], r1[:], 1.0)
                    # comp = 1 - bins
                    nc.vector.tensor_scalar(binsall[nc2][:, d, 0, :],
                                            binsall[nc2][:, d, 1, :],
                                            -1.0, 1.0, ALU.mult, ALU.add)

            # A/B outer-product halves; emitted as closures so chunk 1's
            # builds interleave chunk 0's leaf blocks (keeps PE gaps short)
            Ats, Bts = [None, None], [None, None]

            def _ab_ops(nc2):
                ba = binsall[nc2]

                def f_p01():
                    p01 = osb.tile([128, 2, 2, BL], bf16, tag=f"p01_{nc2}",
                                   name=f"p01_{nc2}")
                    nc.vector.tensor_mul(
                        p01[:],
                        ba[:, 0, :, :].unsqueeze(1).broadcast_to([128, 2, 2, BL]),
                        ba[:, 1, :, :].unsqueeze(2).broadcast_to([128, 2, 2, BL]))
                    return p01

                def f_A(p01):
                    At = osb.tile([128, 2, 4, BL], bf16, tag=f"At_{nc2}",
                                  name=f"At_{nc2}")
                    nc.vector.tensor_mul(
                        At[:],
                        p01[:].rearrange("p a b x -> p (a b) x").unsqueeze(1)
                            .broadcast_to([128, 2, 4, BL]),
                        ba[:, 2, :, :].unsqueeze(2).broadcast_to([128, 2, 4, BL]))
                    Ats[nc2] = At

                def f_p34():
                    p34 = osb.tile([128, 2, 2, BL], bf16, tag=f"p34_{nc2}",
                                   name=f"p34_{nc2}")
                    nc.vector.tensor_mul(
                        p34[:],
                        ba[:, 3, :, :].unsqueeze(1).broadcast_to([128, 2, 2, BL]),
                        ba[:, 4, :, :].unsqueeze(2).broadcast_to([128, 2, 2, BL]))
                    return p34

                def f_B(p34):
                    Bt = osb.tile([128, 2, 4, BL], bf16, tag=f"Bt_{nc2}",
                                  name=f"Bt_{nc2}")
                    nc.vector.tensor_mul(
                        Bt[:],
                        p34[:].rearrange("p a b x -> p (a b) x").unsqueeze(1)
                            .broadcast_to([128, 2, 4, BL]),
                        ba[:, 5, :, :].unsqueeze(2).broadcast_to([128, 2, 4, BL]))
                    Bts[nc2] = Bt

                state = {}
                yield_ops = [
                    lambda: state.__setitem__("p01", f_p01()),
                    lambda: f_A(state["p01"]),
                    lambda: state.__setitem__("p34", f_p34()),
                    lambda: f_B(state["p34"]),
                ]
                return yield_ops

            ab0 = _ab_ops(0)
            ab1 = _ab_ops(1)
            for op in ab0:
                op()

            outp = pout.tile([UNITS, BL], f32, space="PSUM", tag="outp", name="outp")
            ab1_iter = iter(ab1)

            def _leaf_chunk(nc2):
                At_v = Ats[nc2][:].rearrange("p a b x -> p (a b) x")
                Bt_v = Bts[nc2][:].rearrange("p a b x -> p (a b) x")
                for hp in range(4):
                    lb = plf.tile([128, 2, 8, BL], bf16, tag="lb", name="lb")
                    nc.vector.tensor_mul(
                        lb[:],
                        At_v.unsqueeze(1).broadcast_to([128, 2, 8, BL]),
                        Bt_v[:, 2 * hp:2 * hp + 2, :].unsqueeze(2)
                            .broadcast_to([128, 2, 8, BL]))
                    if nc2 == 0 and hp >= 2:
                        # interleave chunk 1's A/B builds into the DVE stream
                        for _ in range(2):
                            try:
                                next(ab1_iter)()
                            except StopIteration:
                                pass
                    for j in range(2):
                        hi = 2 * hp + j
                        for lo in range(8):
                            first = (nc2 == 0 and hi == 0 and lo == 0)
                            last = (nc2 == NCH - 1 and hi == 7 and lo == 7)
                            nc.tensor.matmul(outp[:], rw_s[:, nc2, hi, lo, :],
                                             lb[:, j, lo, :], start=first, stop=last)

            _leaf_chunk(0)
            for op in ab1_iter:
                op()
            _leaf_chunk(1)

            nc.scalar.copy(outs[:], outp[:])
            nc.sync.dma_start(d_out.ap()[:], outs[:])

    nc.compile()
    return nc


def _get_program():
    global _PROG
    if _PROG is None:
        _PROG = _build_program()
    return _PROG


def kernel(inputs, memory, Wq, bq, Wk, bk, Wv, bv, Wo, bo,
           fs_logits, thresholds, log_temp, response):
    x = np.asarray(inputs, np.float32)
    memory = np.asarray(memory, np.float32)
    Wq = np.asarray(Wq, np.float32)
    bq = np.asarray(bq, np.float32)
    Wk = np.asarray(Wk, np.float32)
    bk = np.asarray(bk, np.float32)
    Wv = np.asarray(Wv, np.float32)
    bv = np.asarray(bv, np.float32)
    Wo = np.asarray(Wo, np.float32)
    bo = np.asarray(bo, np.float32)
    fs_logits = np.asarray(fs_logits, np.float32)
    thresholds = np.asarray(thresholds, np.float32)
    log_temp = np.asarray(log_temp, np.float32)
    response = np.asarray(response, np.float32)

    inv_sqrt_k = np.float32(1.0 / np.sqrt(K))
    mem_ext = np.concatenate([np.ones((1, F), np.float32), memory], axis=0)  # [S,F]

    wq_full = np.concatenate([Wq.reshape(F, HK), bq.reshape(1, HK)], axis=0) * inv_sqrt_k
    wk2 = (mem_ext.T[:, :, None] * Wk.reshape(F, 1, HK)).reshape(F, SHK)
    wk_full = np.concatenate([wk2, np.tile(bk.reshape(1, HK), (1, S))], axis=0)
    wqk_full = np.concatenate([wq_full, wk_full], axis=1)     # [F+1, 272]
    wv2 = (mem_ext.T[:, :, None] * Wv.reshape(F, 1, HK)).reshape(F, SHK)
    wv_full = np.concatenate([wv2, np.tile(bv.reshape(1, HK), (1, S))], axis=0)
    wo_full = np.concatenate([Wo.reshape(HK, F), bo.reshape(1, F)], axis=0)

    fs = _sparsemax(fs_logits)                                # [n,d,F]
    a = np.exp(-log_temp).astype(np.float32)                  # [n,d]
    fs_scaled = fs * (np.float32(0.5) * a)[:, :, None]        # [n,d,F]
    # fsw[fr, d, fc, nc2, ncol] = fs_scaled[nc2*128+ncol, d, fc*128+fr]
    fsw = np.ascontiguousarray(
        fs_scaled.transpose(2, 1, 0)                          # [F, d, n]
        .reshape(2, 128, DEPTH, NCH, 128)
        .transpose(1, 2, 0, 3, 4)).astype(BF16)
    c = np.float32(0.5) - np.float32(0.5) * a * thresholds    # [n,d]
    cb = np.ascontiguousarray(
        c.reshape(NCH, 128, DEPTH).transpose(1, 0, 2).reshape(128, NCH * DEPTH)
    ).astype(np.float32)
    rw = np.ascontiguousarray(
        response.reshape(NCH, 128, 8, 8, UNITS).transpose(1, 0, 2, 3, 4)
    ).astype(BF16)
    ident = np.eye(128, dtype=np.float32)

    xT = np.ascontiguousarray(x.T).astype(BF16)               # [F, B]
    ones_row = np.ones((1, BL), BF16)

    nc = _get_program()

    shared = {
        "wqk": np.ascontiguousarray(wqk_full).astype(BF16),
        "wv": np.ascontiguousarray(wv_full).astype(BF16),
        "wo": np.ascontiguousarray(wo_full).astype(BF16),
        "fsw": fsw,
        "cb": cb,
        "rw": rw,
        "ident": ident,
    }
    in_maps = []
    for ci in range(NCORES):
        xta = np.concatenate(
            [np.ascontiguousarray(xT[:, ci * BL:(ci + 1) * BL]), ones_row], axis=0)
        m = dict(shared)
        m["xta"] = xta
        in_maps.append(m)

    from concourse.bass_utils import run_bass_kernel_spmd

    trace = bool(int(os.environ.get("KBENCH_TRACE", "0")))
    tmpdir = os.environ.get("KBENCH_TRACE_DIR") or None
    res = run_bass_kernel_spmd(nc, in_maps, list(range(NCORES)), trace=trace,
                               tmpdir=tmpdir)
    global LAST_EXEC_NS
    LAST_EXEC_NS = res.exec_time_ns

    out = np.concatenate(
        [res.results[ci]["out"].T for ci in range(NCORES)], axis=0)
    return np.ascontiguousarray(out.astype(np.float32))
